# revision 28
# baseline (speedup 1.0000x reference)
"""GNN edge-scorer (MLPPredictor) Trainium2 kernel.

score[e, :] = h[src[e]] @ Wu.T + h[dst[e]] @ Wv.T + b
  h   [100000, 128] f32
  src/dst [600000] i64
  W   [64, 256] f32  (Wu = W[:, :128], Wv = W[:, 128:])
  out [600000, 64] f32

Strategy (8 cores, edge-sharded, fp16 internally):
  - Host: cast h -> fp16 [100096, 128]; per core sort its 75k edges by
    (src>>15, dst>>15) into 16 bins so node indices fit int16 relative to a
    32768-row table base; pad bins to 128 (+last bin to make total %512);
    lay per-chunk indices as int16 [16, L/16] blocks replicated across all
    8 GPSIMD Q7 partition groups in a [128, cols] SBUF canvas.
  - Device (per core): for each 7680-edge slab:
      * bulk non-transpose dma_gather instructions (<=3840 rows each,
        single_packet=False, round-robin over 4 SWDGE queues) pull h16
        rows (256B) into gu/gv [128, 60, 128] fp16, edge i at
        [i%128, i//128, :]; thousands of rows per Pool instruction make
        SWDGE emission cheap, and 4 queues keep 4 descriptor streams in
        flight to hide per-descriptor HBM latency
      * per 512-edge group: 4 PE transposes per side into one fp16
        psum[128, 512], one ACT/DVE copy -> guT/gvT SBUF slab, then
        psum[64, 512] = WuT.T@guT + WvT.T@gvT (+ b via rank-1 matmul),
        ACT copies psum -> fp16 score slab
      * one [64, slab] fp16 store (15 KB/partition descriptors)
  - Host: transpose [64, S_PAD] -> [S_PAD, 64], un-permute, cast f32.
"""

import numpy as np

import concourse.bacc as bacc
import concourse.bass as bass
import concourse.mybir as mybir
import concourse.tile as tile
from concourse.bass_utils import run_bass_kernel_spmd

N_CORES = 8
N_NODES = 100000
N_EDGES = 600000
D = 128
C = 64
N_NODES_PAD = 100096          # mult of 128
EDGES_PER_CORE = N_EDGES // N_CORES
BIN_SHIFT = 15                # 32768-node table windows (int16 idx)
N_SRC_BINS = (N_NODES >> BIN_SHIFT) + 1   # 4
N_BINS = N_SRC_BINS * N_SRC_BINS          # 16
SLAB = 7680                   # edges per score slab / gather target tile
MM = 512                      # edges per matmul group (one PSUM bank)
# One gather needs num_idxs/16 + 2 descriptor-ring slots per engine and the
# ring holds ~1024; 7680 -> 482 slots. Requires single_packet=False (a
# coalesced packet is limited to ~4KB/16 descs per engine).
MAX_GATHER = 3840
# Non-transpose (CME) gathers carry no xbar state, so they can spread across
# all 4 SWDGE queues for drain concurrency. (Transpose-mode gathers CANNOT:
# concurrent rx streams interleave per-descriptor and corrupt the xbar tile.)
N_QUEUES = 4

_F32 = mybir.dt.float32
_F16 = mybir.dt.float16
_I16 = mybir.dt.int16

_CACHE: dict = {}


def _plan_chunks(caps):
    """Static per-core slot plan from bin caps (same caps on all cores).

    Returns (s_pad, slabs, u_chunks, v_chunks, total_idx_cols) where each
    chunk is (slab_idx, dst_off_in_slab, length, table_base_row, idx_col0).
    Chunks never cross slab boundaries and are 128-aligned.
    """
    s_pad = int(sum(caps))
    assert s_pad % MM == 0
    bin_start = np.concatenate([[0], np.cumsum(caps)]).astype(int)

    slabs = []
    off = 0
    while off < s_pad:
        slabs.append(min(SLAB, s_pad - off))
        off += SLAB

    col = [0]

    def split(run_start, run_len, base_row):
        out = []
        pos = run_start
        end = run_start + run_len
        while pos < end:
            slab_i = pos // SLAB
            slab_end = (slab_i + 1) * SLAB
            ln = min(end, slab_end) - pos
            # cap at MAX_GATHER (SWDGE ring capacity), keep 128-aligned
            n_pieces = -(-ln // MAX_GATHER)
            piece = -(-(ln // 128) // n_pieces) * 128
            while ln > 0:
                p_ln = min(piece, ln)
                assert p_ln % 128 == 0 and pos % 128 == 0 and p_ln <= MAX_GATHER
                out.append((slab_i, pos - slab_i * SLAB, p_ln, base_row, col[0]))
                col[0] += p_ln // 16
                pos += p_ln
                ln -= p_ln
        return out

    u_chunks = []
    for bu in range(N_SRC_BINS):
        run_start = bin_start[bu * N_SRC_BINS]
        run_len = bin_start[(bu + 1) * N_SRC_BINS] - run_start
        if run_len:
            u_chunks += split(run_start, run_len, bu << BIN_SHIFT)
    v_chunks = []
    for k in range(N_BINS):
        if caps[k]:
            v_chunks += split(bin_start[k], caps[k], (k % N_SRC_BINS) << BIN_SHIFT)
    return s_pad, slabs, u_chunks, v_chunks, col[0]


def build_nc(caps, use_bias):
    key = (tuple(caps), use_bias)
    if key in _CACHE:
        return _CACHE[key]
    s_pad, slabs, u_chunks, v_chunks, idx_cols = _plan_chunks(caps)

    nc = bacc.Bacc("TRN2", target_bir_lowering=False, num_swdge_queues=N_QUEUES)
    h16 = nc.dram_tensor("h16", [N_NODES_PAD, D], _F16, kind="ExternalInput")
    idx = nc.dram_tensor("idx", [128, idx_cols], _I16, kind="ExternalInput")
    wut = nc.dram_tensor("wut", [D, C], _F16, kind="ExternalInput")
    wvt = nc.dram_tensor("wvt", [D, C], _F16, kind="ExternalInput")
    brow = nc.dram_tensor("brow", [1, C], _F16, kind="ExternalInput")
    out = nc.dram_tensor("out", [C, s_pad], _F16, kind="ExternalOutput")

    by_slab_u = {}
    for ch in u_chunks:
        by_slab_u.setdefault(ch[0], []).append(ch)
    by_slab_v = {}
    for ch in v_chunks:
        by_slab_v.setdefault(ch[0], []).append(ch)

    from concourse.masks import make_identity

    with tile.TileContext(nc) as tc:
        with (
            tc.tile_pool(name="const", bufs=1) as cpool,
            tc.tile_pool(name="gather", bufs=3) as gpool,
            tc.tile_pool(name="gt", bufs=6) as gtpool,
            tc.tile_pool(name="score", bufs=2) as spool,
            tc.tile_pool(name="psum_t", bufs=4, space="PSUM") as ptpool,
            tc.tile_pool(name="psum_s", bufs=2, space="PSUM") as pspool,
        ):
            ident = cpool.tile([128, 128], _F16)
            make_identity(nc, ident[:])
            wut_sb = cpool.tile([D, C], _F16)
            nc.sync.dma_start(out=wut_sb[:], in_=wut[:, :])
            wvt_sb = cpool.tile([D, C], _F16)
            nc.sync.dma_start(out=wvt_sb[:], in_=wvt[:, :])
            idx_sb = cpool.tile([128, idx_cols], _I16)
            # load slab 0's index columns first so the first gathers don't
            # wait on the whole canvas; the rest streams in behind them
            def _cols(chs):
                return (min(c[4] for c in chs),
                        max(c[4] + c[2] // 16 for c in chs))
            first = [_cols(by_slab_u[0]), _cols(by_slab_v[0])]
            rest = []
            prev = 0
            for a, b in sorted(first):
                if prev < a:
                    rest.append((prev, a))
                prev = max(prev, b)
            if prev < idx_cols:
                rest.append((prev, idx_cols))
            for a, b in first + rest:
                nc.sync.dma_start(out=idx_sb[:, a:b], in_=idx[:, a:b])
            if use_bias:
                bias_sb = cpool.tile([1, C], _F16)
                nc.sync.dma_start(out=bias_sb[:], in_=brow[:, :])
                ones_sb = cpool.tile([1, MM], _F16)
                nc.vector.memset(ones_sb[:], 1.0)

            qn = 0
            cp = 0
            for s, slab_len in enumerate(slabs):
                gu = gpool.tile([128, SLAB // 128, D], _F16, tag="gu")
                gv = gpool.tile([128, SLAB // 128, D], _F16, tag="gv")
                for tgt, chunks in ((gu, by_slab_u[s]), (gv, by_slab_v[s])):
                    for (_si, doff, ln, base, c0) in chunks:
                        hi = min(base + (1 << BIN_SHIFT), N_NODES_PAD)
                        nc.gpsimd.dma_gather(
                            tgt[:, doff // 128:(doff + ln) // 128, :],
                            h16[base:hi, :],
                            idx_sb[:, c0:c0 + ln // 16],
                            ln,
                            ln,
                            D,
                            single_packet=False,
                            queue_num=qn,
                        )
                        qn = (qn + 1) % N_QUEUES
                score = spool.tile([C, SLAB], _F16, tag="sc")
                for b in range(slab_len // MM):
                    guT = gtpool.tile([128, MM], _F16, tag="gut")
                    gvT = gtpool.tile([128, MM], _F16, tag="gvt")
                    for g_tile, gT in ((gu, guT), (gv, gvT)):
                        pt = ptpool.tile([128, MM], _F16, tag="pt")
                        for j in range(MM // 128):
                            sub = b * (MM // 128) + j
                            nc.tensor.transpose(
                                pt[:, j * 128:(j + 1) * 128], g_tile[:, sub, :], ident[:]
                            )
                        if cp % 2 == 0:
                            nc.scalar.copy(out=gT[:], in_=pt[:])
                        else:
                            nc.vector.tensor_copy(gT[:], pt[:])
                        cp += 1
                    ps = pspool.tile([C, MM], _F32, tag="ps")
                    nc.tensor.matmul(
                        ps[:], lhsT=wut_sb[:], rhs=guT[:], start=True, stop=False,
                    )
                    nc.tensor.matmul(
                        ps[:], lhsT=wvt_sb[:], rhs=gvT[:],
                        start=False, stop=not use_bias,
                    )
                    if use_bias:
                        nc.tensor.matmul(
                            ps[:], lhsT=bias_sb[:], rhs=ones_sb[:],
                            start=False, stop=True,
                        )
                    nc.scalar.copy(out=score[:, b * MM:(b + 1) * MM], in_=ps[:])
                nc.sync.dma_start(
                    out=out[:, s * SLAB:s * SLAB + slab_len],
                    in_=score[:, :slab_len],
                )

    nc.finalize()
    _CACHE[key] = (nc, s_pad, u_chunks, v_chunks, idx_cols)
    return _CACHE[key]


def _bin_core(src_c, dst_c):
    bu = src_c >> BIN_SHIFT
    bv = dst_c >> BIN_SHIFT
    key = bu * N_SRC_BINS + bv
    order = np.argsort(key, kind="stable")
    counts = np.bincount(key, minlength=N_BINS)
    return order, counts


def make_in_maps(h, src, dst, W, b):
    h = np.asarray(h, dtype=np.float32)
    W = np.asarray(W, dtype=np.float32)
    b = np.asarray(b, dtype=np.float32)
    src = np.asarray(src).astype(np.int64)
    dst = np.asarray(dst).astype(np.int64)

    h16 = np.zeros((N_NODES_PAD, D), dtype=np.float16)
    h16[:N_NODES] = h.astype(np.float16)
    wut = np.ascontiguousarray(W[:, :D].T).astype(np.float16)
    wvt = np.ascontiguousarray(W[:, D:].T).astype(np.float16)
    brow = b.reshape(1, C).astype(np.float16)
    use_bias = bool(np.any(b))

    per_core = []
    max_counts = np.zeros(N_BINS, dtype=np.int64)
    for c in range(N_CORES):
        sl = slice(c * EDGES_PER_CORE, (c + 1) * EDGES_PER_CORE)
        order, counts = _bin_core(src[sl], dst[sl])
        per_core.append((order, counts))
        max_counts = np.maximum(max_counts, counts)

    caps = ((max_counts + 127) // 128) * 128
    total = int(caps.sum())
    pad_tail = (-total) % MM
    caps[-1] += pad_tail  # bin 15 always exists structurally
    caps = caps.astype(int)

    (nc, s_pad, u_chunks, v_chunks, idx_cols) = build_nc(tuple(caps), use_bias)
    bin_start = np.concatenate([[0], np.cumsum(caps)]).astype(int)

    in_maps = []
    orig_ids = []
    for c in range(N_CORES):
        e0 = c * EDGES_PER_CORE
        order, counts = per_core[c]
        src_c = src[e0:e0 + EDGES_PER_CORE][order]
        dst_c = dst[e0:e0 + EDGES_PER_CORE][order]
        key_sorted = (src_c >> BIN_SHIFT) * N_SRC_BINS + (dst_c >> BIN_SHIFT)

        u_slot = np.zeros(s_pad, dtype=np.int16)
        v_slot = np.zeros(s_pad, dtype=np.int16)
        oid = np.full(s_pad, -1, dtype=np.int64)
        pos = 0
        for k in range(N_BINS):
            n_k = int(counts[k])
            b0 = bin_start[k]
            u_slot[b0:b0 + n_k] = (src_c[pos:pos + n_k] & ((1 << BIN_SHIFT) - 1)).astype(np.int16)
            v_slot[b0:b0 + n_k] = (dst_c[pos:pos + n_k] & ((1 << BIN_SHIFT) - 1)).astype(np.int16)
            assert (key_sorted[pos:pos + n_k] == k).all()
            oid[b0:b0 + n_k] = e0 + order[pos:pos + n_k]
            pos += n_k

        canvas = np.zeros((16, idx_cols), dtype=np.int16)
        for chunks, slot in ((u_chunks, u_slot), (v_chunks, v_slot)):
            for (si, doff, ln, _base, c0) in chunks:
                g0 = si * SLAB + doff
                canvas[:, c0:c0 + ln // 16] = slot[g0:g0 + ln].reshape(ln // 16, 16).T
        # idx rows are read per-Q7-core from its own 16-partition group:
        # replicate the block across all 8 groups.
        canvas = np.tile(canvas, (8, 1))

        orig_ids.append(oid)
        in_maps.append(
            {"h16": h16, "idx": canvas, "wut": wut, "wvt": wvt, "brow": brow}
        )
    return nc, in_maps, orig_ids, s_pad


def assemble_output(results, orig_ids):
    final = np.empty((N_EDGES, C), dtype=np.float32)
    for res, oid in zip(results, orig_ids):
        scores = np.asarray(res["out"]).T.astype(np.float32)  # [s_pad, C]
        valid = oid >= 0
        final[oid[valid]] = scores[valid]
    return final


def run(h, src, dst, W, b, **spmd_kwargs):
    nc, in_maps, orig_ids, _ = make_in_maps(h, src, dst, W, b)
    res = run_bass_kernel_spmd(nc, in_maps, core_ids=list(range(N_CORES)), **spmd_kwargs)
    return assemble_output(res.results, orig_ids), res


def kernel(h, src, dst, W, b):
    out, _ = run(h, src, dst, W, b)
    return out
